# revision 60
# baseline (speedup 1.0000x reference)
"""Dense transformer block (ViT-style) on 8 TRN2 NeuronCores.

Sharding: pure data-parallel over batch B=8 (one batch element per core,
weights replicated). Per-core kernel computes the full block on [1024, 768].

Layout strategy ("B-layout"): activations that feed matmul contractions are
kept transposed ([dim, tokens]); LayerNorms run in natural layout
([tokens, dim]) so stats are free-axis reductions. LN gamma/beta are folded
into the downstream weights host-side, so LN on-chip is just
(x - mean) * rsqrt(var+eps). Attention computes S^T = K @ Q^T directly
(softmax denominators via an appended ones-column in the PV matmul), which
avoids transposing the attention matrix entirely.
"""

import functools

import numpy as np
import ml_dtypes

P = 128
T = 8            # token slots per core: 1024 / 128
NTOK = 1024
C = 768
KC = 6           # C / 128
H = 12
HD = 64
HID = 3072
MC_QK = 12       # (2*C) / 128  -> q|k output chunks
MC_FC1 = 24      # HID / 128
B = 8
EPS = 1e-5
N_CORES = 8

_BF16 = ml_dtypes.bfloat16


def _emit(nc, tc, ctx, mybir, bass, tile, make_identity, d, upto="full"):
    """Emit one full block pass. d: dict of DRAM tensor handles.
    upto: 'front' | 'attn' | 'proj' | 'full' — phase-prefix for HW timing
    attribution (partial variants compute garbage but time realistically)."""
    f32 = mybir.dt.float32
    bf16 = mybir.dt.bfloat16
    AF = mybir.ActivationFunctionType
    OP = mybir.AluOpType
    ts = bass.ts
    from concourse.tile_rust import add_dep_helper

    # ---------------- global pools / tiles ----------------
    glob = ctx.enter_context(tc.tile_pool(name="glob", bufs=1))
    stats = ctx.enter_context(tc.tile_pool(name="stats", bufs=4))
    hstream = ctx.enter_context(tc.tile_pool(name="hstream", bufs=3))

    x_s = glob.tile([P, T, C], f32)                 # residual stream (natural)
    ht_s = glob.tile([P, KC, NTOK], bf16)           # LN output transposed
    ot_s = glob.tile([P, KC, NTOK], bf16)           # attention out (transposed)
    wproj_s = glob.tile([P, KC, C], bf16)
    bqk_s = glob.tile([P, MC_QK], f32)
    bfc1_s = glob.tile([P, MC_FC1], f32)
    pbr_s = glob.tile([1, C], bf16)
    fc2br_s = glob.tile([1, C], bf16)
    bvb_s = glob.tile([P, C], bf16)                 # v bias, bcast to 128 parts
    ident_s = glob.tile([P, P], bf16)
    ones1_s = glob.tile([1, P], bf16)
    eps_s = glob.tile([P, 1], f32)
    nc.vector.memset(eps_s, EPS)

    # DMA issue order = need order: x[0] feeds LN slot 0 immediately; wqk/wv
    # gate V and the QK chunks; everything else trails.
    nc.sync.dma_start(out=x_s[:, 0, 0:384], in_=d["x_pt"].ap()[:, 0, 0:384])
    nc.sync.dma_start(out=x_s[:, 0, 384:768],
                      in_=d["x_pt"].ap()[:, 0, 384:768])
    make_identity(nc, ident_s)
    nc.vector.memset(ones1_s, 1.0)

    def layernorm_transpose(pstr_pool, ics, per_slot=None, batch_sqrt=True,
                            copy_eng=None):
        """LN of x_s token-slots `ics` -> transposed into ht_s.
        batch_sqrt: one Sqrt op per call (use when an exp stream is live --
        ACT table switches cost ~1.3us); else per-slot Sqrt (streams better).
        """
        ics = list(ics)
        n = len(ics)

        def stats_for(j, ic, mv_out):
            stt = stats.tile([P, 2, 6], f32, tag="bnst")
            xg = x_s[:, ic, :].rearrange("p (g d) -> p g d", g=2)
            for g in range(2):
                nc.vector.bn_stats(stt[:, g, :], xg[:, g, :])
            nc.vector.bn_aggr(mv_out, stt)

        def finish(j, ic, mean_ap, rstd_ap):
            h_t = hstream.tile([P, C], bf16, tag="hn")
            nc.vector.tensor_scalar(
                out=h_t, in0=x_s[:, ic, :],
                scalar1=mean_ap, scalar2=rstd_ap,
                op0=OP.subtract, op1=OP.mult,
            )
            for kc in range(KC):
                ptr = pstr_pool.tile([P, P], bf16, tag="ptr")
                nc.tensor.transpose(ptr, h_t[:, ts(kc, P)], ident_s)
                if copy_eng is None:
                    nc.vector.tensor_copy(ht_s[:, kc, ts(ic, P)], ptr)
                else:
                    copy_eng.copy(ht_s[:, kc, ts(ic, P)], ptr)
            if per_slot is not None:
                per_slot(ic)

        if batch_sqrt:
            mvb = stats.tile([P, n, 2], f32, tag="bnmv", name="mvb")
            rstdb = stats.tile([P, n], f32, tag="bnrs", name="rstdb")
            for j, ic in enumerate(ics):
                stats_for(j, ic, mvb[:, j, :])
            nc.scalar.activation(rstdb, mvb[:, :, 1], AF.Sqrt, bias=eps_s)
            nc.vector.reciprocal(rstdb, rstdb)
            for j, ic in enumerate(ics):
                finish(j, ic, mvb[:, j, 0:1], rstdb[:, j:j + 1])
        else:
            for j, ic in enumerate(ics):
                mv = stats.tile([P, 2], f32, tag="bnmv")
                rstd = stats.tile([P, 1], f32, tag="bnrs")
                stats_for(j, ic, mv)
                nc.scalar.activation(rstd, mv[:, 1:2], AF.Sqrt, bias=eps_s)
                nc.vector.reciprocal(rstd, rstd)
                finish(j, ic, mv[:, 0:1], rstd)

    # ================= front: LN1+transpose, V, QK =================
    p23 = ctx.enter_context(tc.tile_pool(name="p23", bufs=1))
    qkt_s = p23.tile([P, MC_QK, NTOK], bf16)     # q^T | k^T
    v_s = p23.tile([P, T, H, HD + 1], bf16)      # v natural + ones col
    nc.vector.memset(v_s[:, :, :, HD:HD + 1], 1.0)

    pw1 = ctx.enter_context(tc.tile_pool(name="pw1", bufs=1))
    if True:
        wqk_s = pw1.tile([P, KC, 2 * C], bf16)

        with tc.tile_pool(name="pwv", bufs=1) as pwv, \
             tc.tile_pool(name="pstr1", bufs=2, space="PSUM") as pstr1, \
             tc.tile_pool(name="psv", bufs=2, space="PSUM") as psv:
            wv_s = pwv.tile([P, KC, C], bf16)
            nc.sync.dma_start(out=wv_s, in_=d["wv"].ap())
            for ic in range(1, 4):
                nc.sync.dma_start(out=x_s[:, ic, :],
                                  in_=d["x_pt"].ap()[:, ic, :])
            bvap = d["bv"].ap()
            nc.sync.dma_start(
                out=bvb_s,
                in_=bass.AP(tensor=bvap.tensor, offset=bvap.offset,
                            ap=[[0, P]] + list(bvap.ap[1:])),
            )
            nc.sync.dma_start(out=wqk_s, in_=d["wqk"].ap())
            for ic in range(4, T):
                nc.sync.dma_start(out=x_s[:, ic, :],
                                  in_=d["x_pt"].ap()[:, ic, :])
            nc.sync.dma_start(out=bqk_s, in_=d["bqk"].ap())
            nc.sync.dma_start(out=bfc1_s, in_=d["bfc1"].ap())
            nc.sync.dma_start(out=pbr_s, in_=d["pb"].ap())
            nc.sync.dma_start(out=fc2br_s, in_=d["fc2b"].ap())

            def emit_v(t):
                pv = psv.tile([P, C], f32, tag="pv")
                for kc in range(KC):
                    for ns, nn_ in ((0, 512), (512, 256)):
                        nc.tensor.matmul(
                            pv[:, ns:ns + nn_],
                            ht_s[:, kc, ts(t, P)],
                            wv_s[:, kc, ns:ns + nn_],
                            start=(kc == 0), stop=(kc == KC - 1),
                        )
                # Act copies psum out, Pool folds the bias in-place: keeps
                # DVE (the LN pacer in the front) out of the v epilogue
                vsl = v_s[:, t, :, 0:HD]
                nc.scalar.copy(vsl, pv.rearrange("p (h d) -> p h d", h=H))
                nc.gpsimd.tensor_add(
                    vsl, vsl, bvb_s.rearrange("p (h d) -> p h d", h=H))
            layernorm_transpose(pstr1, range(T), per_slot=emit_v,
                                batch_sqrt=False, copy_eng=nc.scalar)


    # ============ back: attention (both query halves), then proj/LN2/MLP ============
    # (wqk_s stays resident: q/k chunks are computed interleaved with
    # attention half 0 so the exp stream starts early)
    pm = ctx.enter_context(tc.tile_pool(name="pmlp", bufs=1))
    nc.sync.dma_start(out=wproj_s, in_=d["wproj"].ap())
    wfc2_s = pm.tile([P, MC_FC1, C], bf16)
    nc.sync.dma_start(out=wfc2_s, in_=d["wfc2"].ap())

    with tc.tile_pool(name="pattn", bufs=1) as pa, \
         tc.tile_pool(name="ptp", bufs=5) as ptp, \
         tc.tile_pool(name="pss", bufs=2, space="PSUM") as pss, \
         tc.tile_pool(name="pso", bufs=1, space="PSUM") as pso, \
         tc.tile_pool(name="ptr2", bufs=1, space="PSUM") as ptr2:

        exp_insts = []

        def emit_qk(psqk, mc):
            for nh in range(2):
                pq = psqk.tile([P, 512], f32, tag="pq")
                for kc in range(KC):
                    nc.tensor.matmul(
                        pq,
                        wqk_s[:, kc, ts(mc, P)],
                        ht_s[:, kc, ts(nh, 512)],
                        start=(kc == 0), stop=(kc == KC - 1),
                    )
                nc.vector.tensor_scalar_add(
                    qkt_s[:, mc, ts(nh, 512)], pq, bqk_s[:, mc:mc + 1],
                )

        def attn_half(ihalf, pairs, pso_p):
            # Natural-layout PV: o accumulates as [queries(128) x (hd+1)] so
            # the softmax denominator lands per-partition (plain tensor_scalar
            # normalize, no DMA broadcast) and PV uses the full 128-wide PE.
            # The normalized o is transposed back (PE) into ot_s for proj.
            i0 = ihalf * 512
            for pc in (pairs if pairs is not None else range(KC)):
                ptrt = ptr2.tile([P, 4, P], bf16, tag="ptr2")
                for sub in range(2):
                    h = 2 * pc + sub
                    po = sub * HD
                    o_ps = pso_p.tile([P, 4, HD + 1], f32, tag="pso")
                    for jp in range(T // 2):           # jc pairs
                        s_ps = pss.tile([P, 2, 512], f32, tag="pss")
                        for q in range(2):
                            jc = 2 * jp + q
                            nc.tensor.matmul(
                                s_ps[:, q, :],
                                qkt_s[po:po + HD, KC + pc, ts(jc, P)],
                                qkt_s[po:po + HD, pc, i0:i0 + 512],
                                start=True, stop=True,
                            )
                        pt = ptp.tile([P, 2, 512], bf16, tag="pt")
                        exp_insts.append(
                            nc.scalar.activation(pt, s_ps, AF.Exp,
                                                 scale=float(HD) ** -0.5))
                        for q in range(2):
                            jc = 2 * jp + q
                            for tch in range(4):
                                # one start per PSUM bank: the first write
                                # marks the whole 2KB region pending-zero, so
                                # the other tch regions' first writes land as
                                # plain writes (no stale accumulate)
                                nc.tensor.matmul(
                                    o_ps[:, tch, :],
                                    pt[:, q, ts(tch, P)],
                                    v_s[:, jc, h, :],
                                    start=(jp == 0 and q == 0 and tch == 0),
                                    stop=(jp == T // 2 - 1 and q == 1),
                                    skip_group_check=True,
                                )
                    rz = pa.tile([P, 4], f32, tag="rz", bufs=2)
                    nc.vector.reciprocal(rz, o_ps[:, :, HD])
                    for tch in range(4):
                        onat = pa.tile([P, HD], bf16, tag="onat", bufs=4)
                        nc.vector.tensor_scalar_mul(
                            onat, o_ps[:, tch, 0:HD], rz[:, tch:tch + 1])
                        nc.tensor.transpose(
                            ptrt[po:po + HD, tch, :], onat, ident_s)
                nc.vector.tensor_copy(
                    ot_s[:, pc, i0:i0 + 512],
                    ptrt.rearrange("p a b -> p (a b)"))

        with tc.tile_pool(name="psqk", bufs=1, space="PSUM") as psqk:
            for pc in range(KC):
                emit_qk(psqk, pc)
                emit_qk(psqk, KC + pc)
                if upto != "front":
                    attn_half(0, [pc], pso)
        def proj_unit(psx1, ic):
            px = psx1.tile([P, C], f32, tag="px")
            for ns, nn_ in ((0, 512), (512, 256)):
                nc.tensor.matmul(
                    px[:, ns:ns + nn_], ones1_s, pbr_s[:, ns:ns + nn_],
                    start=True, stop=False,
                )
            for kc in range(KC):
                for ns, nn_ in ((0, 512), (512, 256)):
                    nc.tensor.matmul(
                        px[:, ns:ns + nn_],
                        ot_s[:, kc, ts(ic, P)],
                        wproj_s[:, kc, ns:ns + nn_],
                        start=False, stop=(kc == KC - 1),
                    )
            nc.vector.tensor_add(x_s[:, ic, :], x_s[:, ic, :], px)

        w1c_tiles = {}

        def w1c_fetch(mc):
            if mc in w1c_tiles or mc >= MC_FC1:
                return
            w1c = pm.tile([P, KC, P], bf16, tag="w1c", bufs=8)
            nc.sync.dma_start(out=w1c, in_=d["wfc1"].ap()[:, mc, :, :])
            w1c_tiles[mc] = w1c

        def fc1_unit(psg, mc, ihalf, gt_s):
            w1c_fetch(mc)
            w1c_fetch(mc + 4)          # prefetch 4 chunks ahead in the ring
            w1c = w1c_tiles.pop(mc)
            pg = psg.tile([P, 512], f32, tag="pg")
            for kc in range(KC):
                nc.tensor.matmul(
                    pg,
                    w1c[:, kc, :],
                    ht_s[:, kc, ihalf * 512:ihalf * 512 + 512],
                    start=(kc == 0), stop=(kc == KC - 1),
                )
            nc.vector.tensor_scalar_add(
                gt_s[:, mc, :], pg, bfc1_s[:, mc:mc + 1])
            gi = nc.scalar.activation(
                gt_s[:, mc, :], gt_s[:, mc, :], AF.Gelu)
            if exp_insts:
                add_dep_helper(gi.ins, exp_insts[-1].ins, sync=False,
                               reason="batch gelus after exps")

        def fc2_unit(px2p, icl, ihalf, gt_s, tail=False):
            ic = ihalf * 4 + icl
            if tail:
                # last row: finish the 512-chunk first so its add+DMA overlap
                # the 256-chunk matmuls — shortens the end-of-kernel drain
                for ns, nn_ in ((0, 512), (512, 256)):
                    px2 = px2p.tile([P, 512], f32, tag="px2t")
                    pxv = px2[:, 0:nn_]
                    nc.tensor.matmul(pxv, ones1_s, fc2br_s[:, ns:ns + nn_],
                                     start=True, stop=False)
                    for mc in range(MC_FC1):
                        nc.tensor.matmul(
                            pxv,
                            gt_s[:, mc, ts(icl, P)],
                            wfc2_s[:, mc, ns:ns + nn_],
                            start=False, stop=(mc == MC_FC1 - 1),
                        )
                    nc.vector.tensor_add(x_s[:, ic, ns:ns + nn_],
                                         x_s[:, ic, ns:ns + nn_], pxv)
                    nc.sync.dma_start(out=d["out"].ap()[:, ic, ns:ns + nn_],
                                      in_=x_s[:, ic, ns:ns + nn_])
                return
            px2 = px2p.tile([P, C], f32, tag="px2")
            for ns, nn_ in ((0, 512), (512, 256)):
                nc.tensor.matmul(
                    px2[:, ns:ns + nn_], ones1_s, fc2br_s[:, ns:ns + nn_],
                    start=True, stop=False,
                )
            for mc in range(MC_FC1):
                for ns, nn_ in ((0, 512), (512, 256)):
                    nc.tensor.matmul(
                        px2[:, ns:ns + nn_],
                        gt_s[:, mc, ts(icl, P)],
                        wfc2_s[:, mc, ns:ns + nn_],
                        start=False, stop=(mc == MC_FC1 - 1),
                    )
            nc.vector.tensor_add(x_s[:, ic, :], x_s[:, ic, :], px2)
            nc.sync.dma_start(out=d["out"].ap()[:, ic, :],
                              in_=x_s[:, ic, :])

        def proj_half(ihalf, xbufs=1):
            # LN2 of each slot right after its proj unit: spreads the DVE
            # work (px-add, stats, normalize) so PE transposes never wait on
            # a batched DVE backlog
            with tc.tile_pool(name="psx1", bufs=xbufs, space="PSUM") as psx1, \
                 tc.tile_pool(name="pstr2", bufs=2, space="PSUM") as pstr2:
                for icl in range(4):
                    ic = ihalf * 4 + icl
                    proj_unit(psx1, ic)
                    layernorm_transpose(pstr2, (ic,))

        def mlp_half(ihalf, gbufs=2, x2bufs=1, mc0=0, gt_s=None):
            if gt_s is None:
                gt_s = pm.tile([P, MC_FC1, 512], bf16, tag="gt", bufs=1)
            for pmc in range(mc0, min(mc0 + 4, MC_FC1)):
                w1c_fetch(pmc)
            with tc.tile_pool(name="psg", bufs=gbufs, space="PSUM") as psg:
                for mc in range(mc0, MC_FC1):
                    fc1_unit(psg, mc, ihalf, gt_s)
            with tc.tile_pool(name="px2p", bufs=x2bufs, space="PSUM") as px2p:
                for icl in range(4):
                    fc2_unit(px2p, icl, ihalf, gt_s,
                             tail=(ihalf == 1 and icl == 3))

        if upto == "attn":
            attn_half(1, None, pso)
            for ic in range(T):
                nc.sync.dma_start(out=d["out"].ap()[:, ic, :],
                                  in_=x_s[:, ic, :])
            return
        if upto == "proj":
            attn_half(1, None, pso)
            proj_half(0)
            proj_half(1)
            for ic in range(T):
                nc.sync.dma_start(out=d["out"].ap()[:, ic, :],
                                  in_=x_s[:, ic, :])
            return

        # ---- attention half 1 interleaved with proj/LN2/fc1 of half 0 ----
        # PE is in-order: between attention pairs it picks up ready MLP-side
        # work while Act catches up on that pair's exp stream.
        for pc in range(KC):
            attn_half(1, [pc], pso)
            if pc == 0:
                with tc.tile_pool(name="psx1", bufs=1, space="PSUM") as psx1:
                    proj_unit(psx1, 0)
                    proj_unit(psx1, 1)
            elif pc == 1:
                with tc.tile_pool(name="psx1", bufs=1, space="PSUM") as psx1:
                    proj_unit(psx1, 2)
                    proj_unit(psx1, 3)
            elif pc == 2:
                with tc.tile_pool(name="pstr2", bufs=2, space="PSUM") as pstr2:
                    layernorm_transpose(pstr2, (0, 1))
            elif pc == 3:
                with tc.tile_pool(name="pstr2", bufs=2, space="PSUM") as pstr2:
                    layernorm_transpose(pstr2, (2, 3))
            elif pc == 4:
                gt0_s = pm.tile([P, MC_FC1, 512], bf16, tag="gt", bufs=1)
                with tc.tile_pool(name="psg", bufs=1, space="PSUM") as psg:
                    for mc in range(3):
                        fc1_unit(psg, mc, 0, gt0_s)
            else:
                with tc.tile_pool(name="psg", bufs=1, space="PSUM") as psg:
                    for mc in range(3, 6):
                        fc1_unit(psg, mc, 0, gt0_s)
        back_half1 = (proj_half, mlp_half)

    # attention pools are closed now: the rest of the MLP gets deeper PSUM
    # pipelining and doesn't wait on attention bank releases.
    proj_half, mlp_half = back_half1
    mlp_half(0, mc0=6, gt_s=gt0_s, gbufs=3, x2bufs=2)
    proj_half(1, xbufs=2)
    mlp_half(1, gbufs=4, x2bufs=2)


@functools.lru_cache(maxsize=None)
def _build(reps=1, upto="full"):
    from contextlib import ExitStack

    import concourse.bass as bass
    import concourse.mybir as mybir
    import concourse.tile as tile
    from concourse import bacc
    from concourse.masks import make_identity

    f32 = mybir.dt.float32
    bf16 = mybir.dt.bfloat16

    nc = bacc.Bacc("TRN2", target_bir_lowering=False, debug=False,
                   enable_asserts=False)

    d = {
        "x_pt": nc.dram_tensor("x_pt", [P, T, C], f32, kind="ExternalInput"),
        "wqk": nc.dram_tensor("wqk", [P, KC, 2 * C], bf16, kind="ExternalInput"),
        "wv": nc.dram_tensor("wv", [P, KC, C], bf16, kind="ExternalInput"),
        "wproj": nc.dram_tensor("wproj", [P, KC, C], bf16, kind="ExternalInput"),
        "wfc1": nc.dram_tensor("wfc1", [P, MC_FC1, KC, P], bf16, kind="ExternalInput"),
        "wfc2": nc.dram_tensor("wfc2", [P, MC_FC1, C], bf16, kind="ExternalInput"),
        "bqk": nc.dram_tensor("bqk", [P, MC_QK], f32, kind="ExternalInput"),
        "bv": nc.dram_tensor("bv", [1, C], bf16, kind="ExternalInput"),
        "pb": nc.dram_tensor("pb", [1, C], bf16, kind="ExternalInput"),
        "bfc1": nc.dram_tensor("bfc1", [P, MC_FC1], f32, kind="ExternalInput"),
        "fc2b": nc.dram_tensor("fc2b", [1, C], bf16, kind="ExternalInput"),
        "out": nc.dram_tensor("out", [P, T, C], f32, kind="ExternalOutput"),
    }

    with tile.TileContext(nc) as tc:
        for _ in range(reps):
            with ExitStack() as ctx:
                _emit(nc, tc, ctx, mybir, bass, tile, make_identity, d,
                      upto=upto)
    nc.compile()
    return nc


def _to_pt(w, nchunk):
    """[nchunk*128, F] -> [128, nchunk, F] (partition-major chunk layout)."""
    f = w.shape[-1]
    return np.ascontiguousarray(w.reshape(nchunk, P, f).transpose(1, 0, 2))


def _prep_weights(inputs):
    g1 = np.asarray(inputs["ln1_g"], np.float32)
    b1 = np.asarray(inputs["ln1_b"], np.float32)
    g2 = np.asarray(inputs["ln2_g"], np.float32)
    b2 = np.asarray(inputs["ln2_b"], np.float32)
    qkv_w = np.asarray(inputs["qkv_w"], np.float32)
    proj_w = np.asarray(inputs["proj_w"], np.float32)
    proj_b = np.asarray(inputs["proj_b"], np.float32)
    fc1_w = np.asarray(inputs["fc1_w"], np.float32)
    fc1_b = np.asarray(inputs["fc1_b"], np.float32)
    fc2_w = np.asarray(inputs["fc2_w"], np.float32)
    fc2_b = np.asarray(inputs["fc2_b"], np.float32)

    wqk_eff = g1[:, None] * qkv_w[:, :2 * C]
    wv_eff = g1[:, None] * qkv_w[:, 2 * C:]
    bqk = b1 @ qkv_w[:, :2 * C]
    bv = b1 @ qkv_w[:, 2 * C:]
    wfc1_eff = g2[:, None] * fc1_w
    bfc1 = fc1_b + b2 @ fc1_w

    return {
        "wqk": _to_pt(wqk_eff, KC).astype(_BF16),
        "wv": _to_pt(wv_eff, KC).astype(_BF16),
        "wproj": _to_pt(proj_w, KC).astype(_BF16),
        # [c, hid] -> [p=c%128, mc=hid//128, kc=c//128, hid%128]
        "wfc1": np.ascontiguousarray(
            wfc1_eff.reshape(KC, P, MC_FC1, P).transpose(1, 2, 0, 3)
        ).astype(_BF16),
        "wfc2": _to_pt(fc2_w, MC_FC1).astype(_BF16),
        "bqk": np.ascontiguousarray(bqk.reshape(MC_QK, P).T),
        "bv": np.ascontiguousarray(bv.reshape(1, C)).astype(_BF16),
        "pb": np.ascontiguousarray(proj_b.reshape(1, C)).astype(_BF16),
        "bfc1": np.ascontiguousarray(bfc1.reshape(MC_FC1, P).T),
        "fc2b": np.ascontiguousarray(fc2_b.reshape(1, C)).astype(_BF16),
    }


def make_in_maps(**inputs):
    """Build the 8 per-core input maps (exposed for test harnesses)."""
    x = np.asarray(inputs["x"], np.float32)
    wmap = _prep_weights(inputs)
    in_maps = []
    for i in range(N_CORES):
        xi = np.ascontiguousarray(
            x[i].reshape(T, P, C).transpose(1, 0, 2))
        in_maps.append({"x_pt": xi, **wmap})
    return in_maps


def kernel(**inputs):
    from concourse import bass_utils

    nc = _build()
    in_maps = make_in_maps(**inputs)
    res = bass_utils.run_bass_kernel_spmd(nc, in_maps,
                                          core_ids=list(range(N_CORES)))
    outs = [
        np.asarray(r["out"], np.float32).transpose(1, 0, 2).reshape(NTOK, C)
        for r in res.results
    ]
    return np.stack(outs)



# revision 68
# speedup vs baseline: 1.0906x; 1.0906x over previous
"""Dense transformer block (ViT-style) on 8 TRN2 NeuronCores.

Sharding: pure data-parallel over batch B=8 (one batch element per core,
weights replicated). Per-core kernel computes the full block on [1024, 768].

Layout strategy ("B-layout"): activations that feed matmul contractions are
kept transposed ([dim, tokens]); LayerNorms run in natural layout
([tokens, dim]) so stats are free-axis reductions. LN gamma/beta are folded
into the downstream weights host-side, so LN on-chip is just
(x - mean) * rsqrt(var+eps). Attention computes S^T = K @ Q^T directly
(softmax denominators via an appended ones-column in the PV matmul), which
avoids transposing the attention matrix entirely.
"""

import functools

import numpy as np
import ml_dtypes

P = 128
T = 8            # token slots per core: 1024 / 128
NTOK = 1024
C = 768
KC = 6           # C / 128
H = 12
HD = 64
HID = 3072
MC_QK = 12       # (2*C) / 128  -> q|k output chunks
MC_FC1 = 24      # HID / 128
B = 8
EPS = 1e-5
N_CORES = 8

_BF16 = ml_dtypes.bfloat16


def _emit(nc, tc, ctx, mybir, bass, tile, make_identity, d, upto="full"):
    """Emit one full block pass. d: dict of DRAM tensor handles.
    upto: 'front' | 'attn' | 'proj' | 'full' — phase-prefix for HW timing
    attribution (partial variants compute garbage but time realistically)."""
    f32 = mybir.dt.float32
    bf16 = mybir.dt.bfloat16
    AF = mybir.ActivationFunctionType
    OP = mybir.AluOpType
    ts = bass.ts
    from concourse.tile_rust import add_dep_helper

    # ---------------- global pools / tiles ----------------
    glob = ctx.enter_context(tc.tile_pool(name="glob", bufs=1))
    stats = ctx.enter_context(tc.tile_pool(name="stats", bufs=4))
    hstream = ctx.enter_context(tc.tile_pool(name="hstream", bufs=3))

    x_s = glob.tile([P, T, C], f32)                 # residual stream (natural)
    ht_s = glob.tile([P, KC, NTOK], bf16)           # LN output transposed
    ot_s = glob.tile([P, KC, NTOK], bf16)           # attention out (transposed)
    wproj_s = glob.tile([P, KC, C], bf16)
    bqk_s = glob.tile([P, MC_QK], f32)
    bfc1_s = glob.tile([P, MC_FC1], f32)
    pbr_s = glob.tile([1, C], bf16)
    fc2br_s = glob.tile([1, C], bf16)
    bvb_s = glob.tile([P, C], bf16)                 # v bias, bcast to 128 parts
    ident_s = glob.tile([P, P], bf16)
    ones1_s = glob.tile([1, P], bf16)
    eps_s = glob.tile([P, 1], f32)
    nc.vector.memset(eps_s, EPS)

    # DMA issue order = need order: x[0] feeds LN slot 0 immediately; wqk/wv
    # gate V and the QK chunks; everything else trails.
    nc.sync.dma_start(out=x_s[:, 0, 0:384], in_=d["x_pt"].ap()[:, 0, 0:384])
    nc.sync.dma_start(out=x_s[:, 0, 384:768],
                      in_=d["x_pt"].ap()[:, 0, 384:768])
    make_identity(nc, ident_s)
    nc.vector.memset(ones1_s, 1.0)

    def layernorm_transpose(pstr_pool, ics, per_slot=None, batch_sqrt=True,
                            copy_eng=None):
        """LN of x_s token-slots `ics` -> transposed into ht_s.
        batch_sqrt: one Sqrt op per call (use when an exp stream is live --
        ACT table switches cost ~1.3us); else per-slot Sqrt (streams better).
        """
        ics = list(ics)
        n = len(ics)

        def stats_for(j, ic, mv_out):
            stt = stats.tile([P, 2, 6], f32, tag="bnst")
            xg = x_s[:, ic, :].rearrange("p (g d) -> p g d", g=2)
            for g in range(2):
                nc.vector.bn_stats(stt[:, g, :], xg[:, g, :])
            nc.vector.bn_aggr(mv_out, stt)

        def finish(j, ic, mean_ap, rstd_ap):
            h_t = hstream.tile([P, C], bf16, tag="hn")
            nc.vector.tensor_scalar(
                out=h_t, in0=x_s[:, ic, :],
                scalar1=mean_ap, scalar2=rstd_ap,
                op0=OP.subtract, op1=OP.mult,
            )
            for kc in range(KC):
                ptr = pstr_pool.tile([P, P], bf16, tag="ptr")
                nc.tensor.transpose(ptr, h_t[:, ts(kc, P)], ident_s)
                if copy_eng is None:
                    nc.vector.tensor_copy(ht_s[:, kc, ts(ic, P)], ptr)
                else:
                    copy_eng.copy(ht_s[:, kc, ts(ic, P)], ptr)
            if per_slot is not None:
                per_slot(ic)

        if batch_sqrt:
            mvb = stats.tile([P, n, 2], f32, tag="bnmv", name="mvb")
            rstdb = stats.tile([P, n], f32, tag="bnrs", name="rstdb")
            for j, ic in enumerate(ics):
                stats_for(j, ic, mvb[:, j, :])
            nc.scalar.activation(rstdb, mvb[:, :, 1], AF.Sqrt, bias=eps_s)
            nc.vector.reciprocal(rstdb, rstdb)
            for j, ic in enumerate(ics):
                finish(j, ic, mvb[:, j, 0:1], rstdb[:, j:j + 1])
        else:
            for j, ic in enumerate(ics):
                mv = stats.tile([P, 2], f32, tag="bnmv")
                rstd = stats.tile([P, 1], f32, tag="bnrs")
                stats_for(j, ic, mv)
                nc.scalar.activation(rstd, mv[:, 1:2], AF.Sqrt, bias=eps_s)
                nc.vector.reciprocal(rstd, rstd)
                finish(j, ic, mv[:, 0:1], rstd)

    # ================= front: LN1+transpose, V, QK =================
    p23 = ctx.enter_context(tc.tile_pool(name="p23", bufs=1))
    qkt_s = p23.tile([P, MC_QK, NTOK], bf16)     # q^T | k^T
    v_s = p23.tile([P, T, H, HD + 1], bf16)      # v natural + ones col
    nc.vector.memset(v_s[:, :, :, HD:HD + 1], 1.0)

    pw1 = ctx.enter_context(tc.tile_pool(name="pw1", bufs=1))
    if True:
        wqk_s = pw1.tile([P, KC, 2 * C], bf16)

        with tc.tile_pool(name="pwv", bufs=1) as pwv, \
             tc.tile_pool(name="pstr1", bufs=2, space="PSUM") as pstr1, \
             tc.tile_pool(name="psv", bufs=2, space="PSUM") as psv:
            wv_s = pwv.tile([P, KC, C], bf16)
            nc.sync.dma_start(out=wv_s, in_=d["wv"].ap())
            for ic in range(1, 4):
                nc.sync.dma_start(out=x_s[:, ic, :],
                                  in_=d["x_pt"].ap()[:, ic, :])
            bvap = d["bv"].ap()
            nc.sync.dma_start(
                out=bvb_s,
                in_=bass.AP(tensor=bvap.tensor, offset=bvap.offset,
                            ap=[[0, P]] + list(bvap.ap[1:])),
            )
            nc.sync.dma_start(out=wqk_s, in_=d["wqk"].ap())
            for ic in range(4, T):
                nc.sync.dma_start(out=x_s[:, ic, :],
                                  in_=d["x_pt"].ap()[:, ic, :])
            nc.sync.dma_start(out=bqk_s, in_=d["bqk"].ap())
            nc.sync.dma_start(out=bfc1_s, in_=d["bfc1"].ap())
            nc.sync.dma_start(out=pbr_s, in_=d["pb"].ap())
            nc.sync.dma_start(out=fc2br_s, in_=d["fc2b"].ap())

            def emit_v(t):
                pv = psv.tile([P, C], f32, tag="pv")
                for kc in range(KC):
                    for ns, nn_ in ((0, 512), (512, 256)):
                        nc.tensor.matmul(
                            pv[:, ns:ns + nn_],
                            ht_s[:, kc, ts(t, P)],
                            wv_s[:, kc, ns:ns + nn_],
                            start=(kc == 0), stop=(kc == KC - 1),
                        )
                # Act copies psum out, Pool folds the bias in-place: keeps
                # DVE (the LN pacer in the front) out of the v epilogue
                vsl = v_s[:, t, :, 0:HD]
                nc.scalar.copy(vsl, pv.rearrange("p (h d) -> p h d", h=H))
                nc.gpsimd.tensor_add(
                    vsl, vsl, bvb_s.rearrange("p (h d) -> p h d", h=H))
            layernorm_transpose(pstr1, range(T), per_slot=emit_v,
                                batch_sqrt=False, copy_eng=nc.scalar)


    # ============ back: attention (both query halves), then proj/LN2/MLP ============
    # (wqk_s stays resident: q/k chunks are computed interleaved with
    # attention half 0 so the exp stream starts early)
    pm = ctx.enter_context(tc.tile_pool(name="pmlp", bufs=1))
    nc.sync.dma_start(out=wproj_s, in_=d["wproj"].ap())
    wfc2_s = pm.tile([P, MC_FC1, C], bf16)
    nc.sync.dma_start(out=wfc2_s, in_=d["wfc2"].ap())

    with tc.tile_pool(name="pattn", bufs=1) as pa, \
         tc.tile_pool(name="ptp", bufs=5) as ptp, \
         tc.tile_pool(name="pss", bufs=2, space="PSUM") as pss, \
         tc.tile_pool(name="pso", bufs=1, space="PSUM") as pso, \
         tc.tile_pool(name="ptr2", bufs=1, space="PSUM") as ptr2:

        exp_insts = []

        def emit_qk(psqk, mc):
            for nh in range(2):
                pq = psqk.tile([P, 512], f32, tag="pq")
                for kc in range(KC):
                    nc.tensor.matmul(
                        pq,
                        wqk_s[:, kc, ts(mc, P)],
                        ht_s[:, kc, ts(nh, 512)],
                        start=(kc == 0), stop=(kc == KC - 1),
                    )
                nc.vector.tensor_scalar_add(
                    qkt_s[:, mc, ts(nh, 512)], pq, bqk_s[:, mc:mc + 1],
                )

        def attn_half(ihalf, pairs, pso_p):
            # Natural-layout PV: o accumulates as [queries(128) x (hd+1)] so
            # the softmax denominator lands per-partition (plain tensor_scalar
            # normalize, no DMA broadcast) and PV uses the full 128-wide PE.
            # The normalized o is transposed back (PE) into ot_s for proj.
            i0 = ihalf * 512
            for pc in (pairs if pairs is not None else range(KC)):
                ptrt = ptr2.tile([P, 4, P], bf16, tag="ptr2")
                for sub in range(2):
                    h = 2 * pc + sub
                    po = sub * HD
                    o_ps = pso_p.tile([P, 4, HD + 1], f32, tag="pso")
                    for jp in range(T // 2):           # jc pairs
                        s_ps = pss.tile([P, 2, 512], f32, tag="pss")
                        for q in range(2):
                            jc = 2 * jp + q
                            nc.tensor.matmul(
                                s_ps[:, q, :],
                                qkt_s[po:po + HD, KC + pc, ts(jc, P)],
                                qkt_s[po:po + HD, pc, i0:i0 + 512],
                                start=True, stop=True,
                            )
                        pt = ptp.tile([P, 2, 512], bf16, tag="pt")
                        exp_insts.append(
                            nc.scalar.activation(pt, s_ps, AF.Exp,
                                                 scale=float(HD) ** -0.5))
                        for q in range(2):
                            jc = 2 * jp + q
                            for tch in range(4):
                                # one start per PSUM bank: the first write
                                # marks the whole 2KB region pending-zero, so
                                # the other tch regions' first writes land as
                                # plain writes (no stale accumulate)
                                nc.tensor.matmul(
                                    o_ps[:, tch, :],
                                    pt[:, q, ts(tch, P)],
                                    v_s[:, jc, h, :],
                                    start=(jp == 0 and q == 0 and tch == 0),
                                    stop=(jp == T // 2 - 1 and q == 1),
                                    skip_group_check=True,
                                )
                    rz = pa.tile([P, 4], f32, tag="rz", bufs=2)
                    nc.vector.reciprocal(rz, o_ps[:, :, HD])
                    for tch in range(4):
                        onat = pa.tile([P, HD], bf16, tag="onat", bufs=4)
                        nc.vector.tensor_scalar_mul(
                            onat, o_ps[:, tch, 0:HD], rz[:, tch:tch + 1])
                        nc.tensor.transpose(
                            ptrt[po:po + HD, tch, :], onat, ident_s)
                nc.vector.tensor_copy(
                    ot_s[:, pc, i0:i0 + 512],
                    ptrt.rearrange("p a b -> p (a b)"))

        with tc.tile_pool(name="psqk", bufs=1, space="PSUM") as psqk:
            for pc in range(KC):
                emit_qk(psqk, pc)
                emit_qk(psqk, KC + pc)
                if upto != "front":
                    attn_half(0, [pc], pso)
        def proj_unit(psx1, ic):
            px = psx1.tile([P, C], f32, tag="px")
            for ns, nn_ in ((0, 512), (512, 256)):
                nc.tensor.matmul(
                    px[:, ns:ns + nn_], ones1_s, pbr_s[:, ns:ns + nn_],
                    start=True, stop=False,
                )
            for kc in range(KC):
                for ns, nn_ in ((0, 512), (512, 256)):
                    nc.tensor.matmul(
                        px[:, ns:ns + nn_],
                        ot_s[:, kc, ts(ic, P)],
                        wproj_s[:, kc, ns:ns + nn_],
                        start=False, stop=(kc == KC - 1),
                    )
            nc.vector.tensor_add(x_s[:, ic, :], x_s[:, ic, :], px)

        w1c_tiles = {}

        def w1c_fetch(mc):
            if mc in w1c_tiles or mc >= MC_FC1:
                return
            w1c = pm.tile([P, KC, P], bf16, tag="w1c", bufs=8)
            nc.sync.dma_start(out=w1c, in_=d["wfc1"].ap()[:, mc, :, :])
            w1c_tiles[mc] = w1c

        deferred_gelus = []

        def fc1_unit(psg, mc, ihalf, gt_s, defer_gelu=False):
            w1c_fetch(mc)
            w1c_fetch(mc + 4)          # prefetch 4 chunks ahead in the ring
            w1c = w1c_tiles.pop(mc)
            pg = psg.tile([P, 512], f32, tag="pg")
            for kc in range(KC):
                nc.tensor.matmul(
                    pg,
                    w1c[:, kc, :],
                    ht_s[:, kc, ihalf * 512:ihalf * 512 + 512],
                    start=(kc == 0), stop=(kc == KC - 1),
                )
            nc.vector.tensor_scalar_add(
                gt_s[:, mc, :], pg, bfc1_s[:, mc:mc + 1])
            if defer_gelu:
                # don't interleave gelus with attention's exp stream — a
                # Gelu<->Exp table switch costs 1.3us on Act each way
                deferred_gelus.append((gt_s, mc))
                return
            gi = nc.scalar.activation(
                gt_s[:, mc, :], gt_s[:, mc, :], AF.Gelu)
            if exp_insts:
                add_dep_helper(gi.ins, exp_insts[-1].ins, sync=False,
                               reason="batch gelus after exps")

        def fc2_unit(px2p, icl, ihalf, gt_s, tail=False):
            ic = ihalf * 4 + icl
            if tail:
                # last row: finish the 512-chunk first so its add+DMA overlap
                # the 256-chunk matmuls — shortens the end-of-kernel drain
                for ns, nn_ in ((0, 512), (512, 256)):
                    px2 = px2p.tile([P, 512], f32, tag="px2t")
                    pxv = px2[:, 0:nn_]
                    nc.tensor.matmul(pxv, ones1_s, fc2br_s[:, ns:ns + nn_],
                                     start=True, stop=False)
                    for mc in range(MC_FC1):
                        nc.tensor.matmul(
                            pxv,
                            gt_s[:, mc, ts(icl, P)],
                            wfc2_s[:, mc, ns:ns + nn_],
                            start=False, stop=(mc == MC_FC1 - 1),
                        )
                    nc.vector.tensor_add(x_s[:, ic, ns:ns + nn_],
                                         x_s[:, ic, ns:ns + nn_], pxv)
                    nc.sync.dma_start(out=d["out"].ap()[:, ic, ns:ns + nn_],
                                      in_=x_s[:, ic, ns:ns + nn_])
                return
            px2 = px2p.tile([P, C], f32, tag="px2")
            for ns, nn_ in ((0, 512), (512, 256)):
                nc.tensor.matmul(
                    px2[:, ns:ns + nn_], ones1_s, fc2br_s[:, ns:ns + nn_],
                    start=True, stop=False,
                )
            for mc in range(MC_FC1):
                for ns, nn_ in ((0, 512), (512, 256)):
                    nc.tensor.matmul(
                        px2[:, ns:ns + nn_],
                        gt_s[:, mc, ts(icl, P)],
                        wfc2_s[:, mc, ns:ns + nn_],
                        start=False, stop=(mc == MC_FC1 - 1),
                    )
            nc.vector.tensor_add(x_s[:, ic, :], x_s[:, ic, :], px2)
            nc.sync.dma_start(out=d["out"].ap()[:, ic, :],
                              in_=x_s[:, ic, :])

        def proj_half(ihalf, xbufs=1):
            # LN2 of each slot right after its proj unit: spreads the DVE
            # work (px-add, stats, normalize) so PE transposes never wait on
            # a batched DVE backlog
            with tc.tile_pool(name="psx1", bufs=xbufs, space="PSUM") as psx1, \
                 tc.tile_pool(name="pstr2", bufs=2, space="PSUM") as pstr2:
                for icl in range(4):
                    ic = ihalf * 4 + icl
                    proj_unit(psx1, ic)
                    layernorm_transpose(pstr2, (ic,))

        def mlp_half(ihalf, gbufs=2, x2bufs=1, mc0=0, gt_s=None):
            if gt_s is None:
                gt_s = pm.tile([P, MC_FC1, 512], bf16, tag="gt", bufs=1)
            for (g_t, g_mc) in deferred_gelus:
                nc.scalar.activation(g_t[:, g_mc, :], g_t[:, g_mc, :],
                                     AF.Gelu)
            deferred_gelus.clear()
            for pmc in range(mc0, min(mc0 + 4, MC_FC1)):
                w1c_fetch(pmc)
            with tc.tile_pool(name="psg", bufs=gbufs, space="PSUM") as psg:
                for mc in range(mc0, MC_FC1):
                    fc1_unit(psg, mc, ihalf, gt_s)
            with tc.tile_pool(name="px2p", bufs=x2bufs, space="PSUM") as px2p:
                for icl in range(4):
                    fc2_unit(px2p, icl, ihalf, gt_s,
                             tail=(ihalf == 1 and icl == 3))

        if upto == "attn":
            attn_half(1, None, pso)
            for ic in range(T):
                nc.sync.dma_start(out=d["out"].ap()[:, ic, :],
                                  in_=x_s[:, ic, :])
            return
        if upto == "proj":
            attn_half(1, None, pso)
            proj_half(0)
            proj_half(1)
            for ic in range(T):
                nc.sync.dma_start(out=d["out"].ap()[:, ic, :],
                                  in_=x_s[:, ic, :])
            return

        # ---- attention half 1 interleaved with proj/LN2/fc1 of half 0 ----
        # PE is in-order: between attention pairs it picks up ready MLP-side
        # work while Act catches up on that pair's exp stream.
        for pc in range(KC):
            attn_half(1, [pc], pso)
            if pc == 0:
                with tc.tile_pool(name="psx1", bufs=1, space="PSUM") as psx1:
                    proj_unit(psx1, 0)
                    proj_unit(psx1, 1)
            elif pc == 1:
                with tc.tile_pool(name="psx1", bufs=1, space="PSUM") as psx1:
                    proj_unit(psx1, 2)
                    proj_unit(psx1, 3)
            elif pc == 2:
                with tc.tile_pool(name="pstr2", bufs=2, space="PSUM") as pstr2:
                    layernorm_transpose(pstr2, (0, 1))
            elif pc == 3:
                with tc.tile_pool(name="pstr2", bufs=2, space="PSUM") as pstr2:
                    layernorm_transpose(pstr2, (2, 3))
            elif pc == 4:
                gt0_s = pm.tile([P, MC_FC1, 512], bf16, tag="gt", bufs=1)
                with tc.tile_pool(name="psg", bufs=1, space="PSUM") as psg:
                    for mc in range(3):
                        fc1_unit(psg, mc, 0, gt0_s)
            else:
                with tc.tile_pool(name="psg", bufs=1, space="PSUM") as psg:
                    for mc in range(3, 6):
                        fc1_unit(psg, mc, 0, gt0_s)
        back_half1 = (proj_half, mlp_half)

    # attention pools are closed now: the rest of the MLP gets deeper PSUM
    # pipelining and doesn't wait on attention bank releases.
    proj_half, mlp_half = back_half1
    mlp_half(0, mc0=6, gt_s=gt0_s, gbufs=3, x2bufs=2)
    proj_half(1, xbufs=2)
    mlp_half(1, gbufs=4, x2bufs=2)


@functools.lru_cache(maxsize=None)
def _build(reps=1, upto="full"):
    from contextlib import ExitStack

    import concourse.bass as bass
    import concourse.mybir as mybir
    import concourse.tile as tile
    from concourse import bacc
    from concourse.masks import make_identity

    f32 = mybir.dt.float32
    bf16 = mybir.dt.bfloat16

    nc = bacc.Bacc("TRN2", target_bir_lowering=False, debug=False,
                   enable_asserts=False)

    d = {
        "x_pt": nc.dram_tensor("x_pt", [P, T, C], f32, kind="ExternalInput"),
        "wqk": nc.dram_tensor("wqk", [P, KC, 2 * C], bf16, kind="ExternalInput"),
        "wv": nc.dram_tensor("wv", [P, KC, C], bf16, kind="ExternalInput"),
        "wproj": nc.dram_tensor("wproj", [P, KC, C], bf16, kind="ExternalInput"),
        "wfc1": nc.dram_tensor("wfc1", [P, MC_FC1, KC, P], bf16, kind="ExternalInput"),
        "wfc2": nc.dram_tensor("wfc2", [P, MC_FC1, C], bf16, kind="ExternalInput"),
        "bqk": nc.dram_tensor("bqk", [P, MC_QK], f32, kind="ExternalInput"),
        "bv": nc.dram_tensor("bv", [1, C], bf16, kind="ExternalInput"),
        "pb": nc.dram_tensor("pb", [1, C], bf16, kind="ExternalInput"),
        "bfc1": nc.dram_tensor("bfc1", [P, MC_FC1], f32, kind="ExternalInput"),
        "fc2b": nc.dram_tensor("fc2b", [1, C], bf16, kind="ExternalInput"),
        "out": nc.dram_tensor("out", [P, T, C], f32, kind="ExternalOutput"),
    }

    with tile.TileContext(nc) as tc:
        for _ in range(reps):
            with ExitStack() as ctx:
                _emit(nc, tc, ctx, mybir, bass, tile, make_identity, d,
                      upto=upto)
    nc.compile()
    return nc


def _to_pt(w, nchunk):
    """[nchunk*128, F] -> [128, nchunk, F] (partition-major chunk layout)."""
    f = w.shape[-1]
    return np.ascontiguousarray(w.reshape(nchunk, P, f).transpose(1, 0, 2))


def _prep_weights(inputs):
    g1 = np.asarray(inputs["ln1_g"], np.float32)
    b1 = np.asarray(inputs["ln1_b"], np.float32)
    g2 = np.asarray(inputs["ln2_g"], np.float32)
    b2 = np.asarray(inputs["ln2_b"], np.float32)
    qkv_w = np.asarray(inputs["qkv_w"], np.float32)
    proj_w = np.asarray(inputs["proj_w"], np.float32)
    proj_b = np.asarray(inputs["proj_b"], np.float32)
    fc1_w = np.asarray(inputs["fc1_w"], np.float32)
    fc1_b = np.asarray(inputs["fc1_b"], np.float32)
    fc2_w = np.asarray(inputs["fc2_w"], np.float32)
    fc2_b = np.asarray(inputs["fc2_b"], np.float32)

    wqk_eff = g1[:, None] * qkv_w[:, :2 * C]
    wv_eff = g1[:, None] * qkv_w[:, 2 * C:]
    bqk = b1 @ qkv_w[:, :2 * C]
    bv = b1 @ qkv_w[:, 2 * C:]
    wfc1_eff = g2[:, None] * fc1_w
    bfc1 = fc1_b + b2 @ fc1_w

    return {
        "wqk": _to_pt(wqk_eff, KC).astype(_BF16),
        "wv": _to_pt(wv_eff, KC).astype(_BF16),
        "wproj": _to_pt(proj_w, KC).astype(_BF16),
        # [c, hid] -> [p=c%128, mc=hid//128, kc=c//128, hid%128]
        "wfc1": np.ascontiguousarray(
            wfc1_eff.reshape(KC, P, MC_FC1, P).transpose(1, 2, 0, 3)
        ).astype(_BF16),
        "wfc2": _to_pt(fc2_w, MC_FC1).astype(_BF16),
        "bqk": np.ascontiguousarray(bqk.reshape(MC_QK, P).T),
        "bv": np.ascontiguousarray(bv.reshape(1, C)).astype(_BF16),
        "pb": np.ascontiguousarray(proj_b.reshape(1, C)).astype(_BF16),
        "bfc1": np.ascontiguousarray(bfc1.reshape(MC_FC1, P).T),
        "fc2b": np.ascontiguousarray(fc2_b.reshape(1, C)).astype(_BF16),
    }


def make_in_maps(**inputs):
    """Build the 8 per-core input maps (exposed for test harnesses)."""
    x = np.asarray(inputs["x"], np.float32)
    wmap = _prep_weights(inputs)
    in_maps = []
    for i in range(N_CORES):
        xi = np.ascontiguousarray(
            x[i].reshape(T, P, C).transpose(1, 0, 2))
        in_maps.append({"x_pt": xi, **wmap})
    return in_maps


def kernel(**inputs):
    from concourse import bass_utils

    nc = _build()
    in_maps = make_in_maps(**inputs)
    res = bass_utils.run_bass_kernel_spmd(nc, in_maps,
                                          core_ids=list(range(N_CORES)))
    outs = [
        np.asarray(r["out"], np.float32).transpose(1, 0, 2).reshape(NTOK, C)
        for r in res.results
    ]
    return np.stack(outs)

